# revision 28
# baseline (speedup 1.0000x reference)
"""Trainium2 Bass kernel for nn_CBSA_45389214384209 (sparse_attention).

Reference computation (per batch element b of 8):
  x_seq = x[b].T                      # [4096, 256]   (x[b] is [256, 4096])
  proj  = x_seq @ W_proj              # [4096, 512]
  rep   = avgpool8x8(proj)            # [64, 512]  == avgpool8x8(x_seq) @ W_proj
  per head h (8 heads, dh=64):
    S    = rep_h @ proj_h.T * scale   # [64, 4096]
    P    = softmax(S)                 # [64, 4096]
    rd   = P @ proj_h                 # [64, 64]
    rep2 = rep_h + step_rep[h] * rd
    P2   = softmax(rep2 @ rep2.T * scale)
    xd2  = step_x[h] * (P2 @ rep2)    # [64, 64]
    xdT  = xd2.T @ P                  # [64, 4096]
  out[b] = W_out.T @ concat_h(xdT) + b_out[:, None]   # [256, 4096]

Sharding: pure data parallel - one batch element per NeuronCore (8 cores).

Restructured to avoid ever materializing proj/projT (the [512, 4096]
projection) on chip:
  - scores:    S = (rep_bd @ W^T) @ x = QW^T @ x, with QW = W @ rep_bd a
               tiny [256, 512] matrix (rep_bd is the block-diagonal
               scaled pooled-query matrix).  Pooling runs on x directly
               (pooling commutes with the projection).
  - values:    rd = P @ (x_seq @ W) = (P @ x_seq) @ W = R @ W, where
               R = P @ x_seq is [512q, 256] per pair-packed layout.  R is
               computed from a DMA-xbar-transposed copy of x, and the
               [64, 64] rd comes from an 8-matmul pass over R.
  - output:    out = W_out.T @ (xd2.T @ P) is accumulated directly as
               sum_h (xd2_h @ W_out_h).T @ P_h into the output PSUM tile
               (no xdT intermediate), with b_out folded into the
               PSUM-drain copy and step_x / step_rep / softmax 1/Z
               folded into small weight/broadcast tensors.
Head pairs are packed into 128-row tiles via block-diagonal operands.
All big matmuls run in bf16 (scores are ~N(0, 1/64); bf16 rounding is
far below the fp32 envelope); softmax is exact in fp32 PSUM.
"""

import os
import sys

import numpy as np

for _p in ("/opt/trn_rl_repo", os.path.expanduser("~/.axon_site/_ro/trn_rl_repo")):
    if os.path.isdir(_p) and _p not in sys.path:
        sys.path.insert(0, _p)

import concourse.bass as bass
import concourse.tile as tile
from concourse import bacc, mybir
from concourse.bass import ds, ts
from concourse.masks import make_identity

F32 = mybir.dt.float32
BF16 = mybir.dt.bfloat16
AX = mybir.AxisListType
ALU = mybir.AluOpType
ACTF = mybir.ActivationFunctionType

B = 8
C = 256          # model dim
T = 4096         # tokens (64x64 grid)
INNER = 512
HEADS = 8
DH = 64
NB = 64          # pooled tokens (8x8 grid)
SCALE = DH ** -0.5
NPAIR = 4        # head pairs
NCHUNK = 8       # 512-wide token chunks
NTT = 32         # 128-wide token tiles

CFG = {"ver": "r2"}


def build_module(cfg=CFG):
    nc = bacc.Bacc("TRN2", debug=False)

    x = nc.dram_tensor("x", [C, T], F32, kind="ExternalInput").ap()
    wp = nc.dram_tensor("w_proj", [C, INNER], F32, kind="ExternalInput").ap()
    wo = nc.dram_tensor("w_out", [INNER, C], F32, kind="ExternalInput").ap()
    bo = nc.dram_tensor("b_out", [C], F32, kind="ExternalInput").ap()
    srep = nc.dram_tensor("s_rep", [HEADS], F32, kind="ExternalInput").ap()
    sx = nc.dram_tensor("s_x", [HEADS], F32, kind="ExternalInput").ap()
    out = nc.dram_tensor("out", [C, T], F32, kind="ExternalOutput").ap()

    with tile.TileContext(nc) as tc:
        _body(tc, cfg, x, wp, wo, bo, srep, sx, out)
    nc.compile()
    return nc


def _body(tc, cfg, x, wp, wo, bo, srep, sx, out):
    nc = tc.nc

    x_r = x.rearrange("(o p) t -> p o t", p=128)      # [128, 2, 4096]
    wp_r = wp.rearrange("(o p) i -> p o i", p=128)    # [128, 2, 512]
    wo_r = wo.rearrange("(g p) c -> p g c", p=128)    # [128, 4, 256]
    bo_r = bo.rearrange("(o p) -> p o", p=128)        # [128, 2]
    out_r = out.rearrange("(o p) t -> p o t", p=128)  # [128, 2, 4096]

    # ---- pools (SBUF pools stack-nested: alloc order == reverse release) --
    consts = tc.alloc_tile_pool(name="consts", bufs=1)
    outp = tc.alloc_tile_pool(name="outp", bufs=3)       # out staging + xd2w
    pp = tc.alloc_tile_pool(name="pp", bufs=1)           # P (attn) tiles
    s4 = tc.alloc_tile_pool(name="s4", bufs=1)           # stage 1.5/2 temps
    ptp = tc.alloc_tile_pool(name="ptp", bufs=1)         # P^T
    xbp = tc.alloc_tile_pool(name="xbp", bufs=1)         # x (bf16) + x^T

    psum0 = tc.alloc_tile_pool(name="psum0", bufs=1, space="PSUM")

    xfp = tc.alloc_tile_pool(name="xfp", bufs=1)         # fp32 x (scores rhs)

    # ---- critical-path loads first: wp (small), then x (HWDGE, fp32) ----
    # Scores consume x directly as float32r (bitcast view of the fp32
    # load); the bf16 copy (only needed for the xbar transposes) is made
    # by ACT per chunk while it is otherwise idle.
    wp_f32 = consts.tile([128, 2, INNER], F32, name="wp_f32")
    nc.sync.dma_start(wp_f32, wp_r)
    wp_bf = consts.tile([128, 2, INNER], BF16, name="wp_bf")
    nc.scalar.copy(wp_bf, wp_f32)

    ident_bf = consts.tile([128, 128], BF16, name="ident_bf")
    make_identity(nc, ident_bf)
    ident_f = consts.tile([128, 128], F32, name="ident_f")
    make_identity(nc, ident_f)
    # warm the Exp activation table while the DMAs run
    act_warm = consts.tile([1, 1], F32, name="act_warm")
    nc.vector.memset(act_warm, 0.0)
    nc.scalar.activation(
        out=act_warm, in_=act_warm, func=ACTF.Exp, bias=0.0, scale=1.0
    )
    ones_f = consts.tile([1, 128], F32, name="ones_f")
    nc.vector.memset(ones_f, 1.0)
    ones_bf = consts.tile([1, 128], BF16, name="ones_bf")
    nc.vector.memset(ones_bf, 1.0)
    # step_rep as a [1, 512] bf16 row (value i -> step_rep[i // 64]), used
    # to build a rank-1 broadcast that scales W_proj columns.
    srep_ld = consts.tile([1, 8], BF16, name="srep_ld")
    srep_flat = bass.AP(
        tensor=srep.tensor, offset=srep.offset,
        ap=[[0, 1], [srep.ap[0][0], 8]],
    )
    nc.gpsimd.dma_start(srep_ld, srep_flat)
    srep_row = consts.tile([1, 8, DH], BF16, name="srep_row")
    nc.vector.tensor_copy(
        srep_row, srep_ld.unsqueeze(2).to_broadcast((1, 8, DH))
    )

    # ---- x: chunked fp32 load + ACT bf16 casts + pooling + transposes ---
    # t = hb*512 + hl*64 + wb*8 + wi ; block index = (hb, wb).  Chunks are
    # sized so the last per-o pooling reduce is short (it gates QW).
    x_f32 = xfp.tile([128, 2, T], F32, name="x_f32")
    x_bf = xbp.tile([128, 2, T], BF16, name="x_bf")
    xT = xbp.tile([128, NTT, 2, 128], BF16, name="xT")
    xp1 = consts.tile([128, 2, 8, 8, 8], BF16, name="xp1")
    xpool = consts.tile([128, 2, 8, 8], BF16, name="xpool")
    CH = ((0, 4), (4, 3), (7, 1))  # (hb0, nhb) chunks per o
    with nc.allow_low_precision("8-deep pooling partial sums in bf16"):
        for o in range(2):
            for hb0, nhb in CH:
                sl = ds(512 * hb0, 512 * nhb)
                nc.sync.dma_start(x_f32[:, o, sl], x_r[:, o, sl])
                nc.scalar.copy(x_bf[:, o, sl], x_f32[:, o, sl])
                nc.vector.reduce_sum(
                    xp1[:, o, ds(hb0, nhb)],
                    x_f32[:, o, sl].rearrange(
                        "p (hb hl wb wi) -> p hb hl wb wi",
                        hb=nhb, hl=8, wb=8, wi=8,
                    ),
                    axis=AX.X,
                )
            nc.vector.reduce_sum(
                xpool[:, o],
                xp1[:, o].rearrange("p hb hl wb -> p hb wb hl"),
                axis=AX.X,
            )

    xfp.release()

    # ---- W^T via PE transposes: [512i, 256c] as [128ip, 4gi, 2o, 128c] --
    wpT_sb = consts.tile([128, 4, 2, 128], BF16, name="wpT_sb")
    for g in range(4):
        wpt_ps = psum0.tile([128, 2, 128], BF16, name="wpt_ps", tag="tp", bufs=2)
        for o in range(2):
            nc.tensor.transpose(wpt_ps[:, o, :], wp_bf[:, o, ts(g, 128)], ident_bf)
        nc.vector.tensor_copy(wpT_sb[:, g, :, :], wpt_ps)

    # ---- x^T via PE transposes (PE is idle during the prologue) ---------
    for o in range(2):
        for g4 in range(8):
            xt_ps = psum0.tile(
                [128, 4, 128], BF16, name="xt_ps", tag="tp", bufs=2
            )
            for c4 in range(4):
                tt = 4 * g4 + c4
                nc.tensor.transpose(
                    xt_ps[:, c4, :], x_bf[:, o, ts(tt, 128)], ident_bf
                )
            if g4 % 2 == 0:
                nc.scalar.copy(xT[:, ds(4 * g4, 4), o, :], xt_ps)
            else:
                nc.vector.tensor_copy(xT[:, ds(4 * g4, 4), o, :], xt_ps)

    # ---- repT (pooled queries): [128i, 4g, 64blk] ----------------------
    rep_ps = psum0.tile([128, 4, NB], F32, name="rep_ps", tag="rep", bufs=1)
    for g in range(4):
        for o in range(2):
            nc.tensor.matmul(
                rep_ps[:, g, :], wp_bf[:, o, ts(g, 128)], xpool[:, o],
                start=(o == 0), stop=(o == 1),
            )
    # block-diagonal scaled copy: one K=128 matmul covers a head pair
    repT_bd = consts.tile([128, 4, 128], BF16, name="repT_bd")
    nc.vector.memset(repT_bd, 0.0)
    for h in range(2):
        rows = slice(64 * h, 64 * h + 64)
        nc.vector.tensor_scalar_mul(
            repT_bd[rows, :, ds(64 * h, 64)], rep_ps[rows, :, :], SCALE / NB
        )

    # ---- QW = W @ rep_bd : [128c, 2cs, 4p, 128q] ------------------------
    qw_ps = psum0.tile([128, 2, 4, 128], F32, name="qw_ps", tag="qw", bufs=1)
    for cs in range(2):
        for p in range(NPAIR):
            nc.tensor.matmul(
                qw_ps[:, cs, p, :], wpT_sb[:, p, cs, :], repT_bd[:, p, :],
                start=True, stop=True,
            )
    QW_sb = consts.tile([128, 2, 4, 128], BF16, name="QW_sb")
    nc.vector.tensor_copy(QW_sb, qw_ps)
    repT_f = consts.tile([128, 4, NB], F32, name="repT_f")
    nc.vector.tensor_scalar_mul(repT_f, rep_ps, 1.0 / NB)
    srep_ps = psum0.tile([128, INNER], F32, name="srep_ps", tag="rep", bufs=1)
    nc.tensor.matmul(srep_ps, ones_bf, srep_row, start=True, stop=True)
    wps_bf = consts.tile([128, 2, INNER], BF16, name="wps_bf")
    for o in range(2):
        nc.vector.tensor_tensor(
            wps_bf[:, o, :], wp_f32[:, o, :], srep_ps, ALU.mult
        )
    psum0.release()

    psumA = tc.alloc_tile_pool(name="psumA", bufs=1, space="PSUM")

    # ---- non-critical constants (consumed in later phases) ---------------
    wo_f32 = consts.tile([128, 4, C], F32, name="wo_f32")
    nc.sync.dma_start(wo_f32, wo_r)
    bo_ld = consts.tile([128, 2], F32, name="bo_ld")
    nc.sync.dma_start(bo_ld, bo_r)
    bo_sb = consts.tile([128, 2], F32, name="bo_sb")
    nc.vector.tensor_copy(bo_sb, bo_ld)
    # step_x as [128, 4] (p, g) -> step_x[2g + p // 64], scales W_out rows.
    sx_map_ld = consts.tile([128, 4], F32, name="sx_map_ld")
    st = sx.ap[0][0]
    for h in range(2):
        half = bass.AP(
            tensor=sx.tensor, offset=sx.offset + h * st,
            ap=[[0, 64], [2 * st, 4]],
        )
        nc.sync.dma_start(sx_map_ld[ds(64 * h, 64), :], half)
    sx_map = consts.tile([128, 4], F32, name="sx_map")
    nc.vector.tensor_copy(sx_map, sx_map_ld)
    wos_bf = consts.tile([128, 4, C], BF16, name="wos_bf")
    for g in range(4):
        nc.vector.tensor_tensor(
            wos_bf[:, g, :], wo_f32[:, g, :],
            sx_map[:, g : g + 1].to_broadcast((128, C)), ALU.mult,
        )

    # ---- stage 1: scores + exp + P^T, pipelined per head pair -----------
    # P_sb[p]: [128, T] unnormalized exp(S); rows 0-63 = head 2p, rows
    # 64-127 = head 2p+1.  Scores are bounded (|s| ~< 2, pooled queries),
    # so exp needs no max-subtraction; 1/Z is folded in later.
    zpart = consts.tile([128, 4, 4], F32, name="zpart")
    p_tiles = [
        pp.tile([128, T], BF16, name=f"p{p}", tag=f"p{p}") for p in range(NPAIR)
    ]
    PT_sb = ptp.tile([128, NTT, 4, 128], BF16, name="PT_sb")

    def scores(p, jb):
        s_ps = psumA.tile([128, 2, 512], F32, name="s_ps", tag="s", bufs=2)
        for o in range(2):
            for j2 in range(2):
                nc.tensor.matmul(
                    s_ps[:, j2, :], QW_sb[:, o, p, :],
                    x_bf[:, o, ts(2 * jb + j2, 512)],
                    start=(o == 0), stop=(o == 1),
                )
        # one 1024-wide exp per psum tile (reads across both banks)
        nc.scalar.activation(
            out=p_tiles[p][:, ts(jb, 1024)], in_=s_ps,
            func=ACTF.Exp, bias=0.0, scale=1.0,
            accum_out=zpart[:, p, jb : jb + 1],
        )

    r_ps = [
        psumA.tile([128, 512], F32, name=f"r_ps{cs}", tag=f"r{cs}", bufs=1)
        for cs in range(2)
    ]

    def r_octet(jb):
        # R^T = x_seq^T @ P^T accumulated over the t tiles of round jb
        for tt in range(8 * jb, 8 * jb + 8):
            for cs in range(2):
                nc.tensor.matmul(
                    r_ps[cs], xT[:, tt, cs, :], PT_sb[:, tt, :, :],
                    start=(tt == 0), stop=(tt == NTT - 1),
                )

    for jb in range(4):
        for p in range(NPAIR):
            scores(p, jb)
        for p in range(NPAIR):
            for half in range(2):
                tp_ps = psumA.tile(
                    [128, 4, 128], BF16, name="tp_ps", tag="tp", bufs=2
                )
                for c4 in range(4):
                    tt = 8 * jb + 4 * half + c4
                    nc.tensor.transpose(
                        tp_ps[:, c4, :], p_tiles[p][:, ts(tt, 128)], ident_bf
                    )
                if (2 * p + half) % 3 == 2:
                    nc.scalar.copy(
                        PT_sb[:, ds(8 * jb + 4 * half, 4), p, :], tp_ps
                    )
                else:
                    nc.vector.tensor_copy(
                        PT_sb[:, ds(8 * jb + 4 * half, 4), p, :], tp_ps
                    )
        if jb > 0:
            r_octet(jb - 1)

    # ---- 1/Z as a rank-1 broadcast rz_bc[i, p, q] = 1/Z[q of pair p] ----
    # (emitted before the final R octet so the small-op chain overlaps it)
    zsum = s4.tile([128, 4], F32, name="zsum")
    nc.vector.reduce_sum(zsum, zpart, axis=AX.X)
    rz = s4.tile([128, 4], F32, name="rz")
    nc.vector.reciprocal(rz, zsum)
    rzt_ps = psumA.tile([1, 4, 128], F32, name="rzt_ps", tag="tp", bufs=2)
    for p in range(NPAIR):
        nc.tensor.transpose(rzt_ps[:, p, :], rz[:, p : p + 1], ident_f)
    rzt_sb = s4.tile([1, 4, 128], F32, name="rzt_sb")
    nc.vector.tensor_copy(rzt_sb, rzt_ps)
    rzbc_ps = psumA.tile([128, 512], F32, name="rzbc_ps", tag="tp", bufs=2)
    nc.tensor.matmul(rzbc_ps, ones_f, rzt_sb, start=True, stop=True)
    rz_bc = s4.tile([128, 4, 128], F32, name="rz_bc")
    nc.vector.tensor_copy(rz_bc, rzbc_ps)

    r_octet(3)

    R_bf = s4.tile([128, 2, 512], BF16, name="R_bf")
    nc.vector.tensor_copy(R_bf[:, 0, :], r_ps[0])
    nc.scalar.copy(R_bf[:, 1, :], r_ps[1])
    xbp.release()
    ptp.release()
    psumA.release()

    psumB = tc.alloc_tile_pool(name="psumB", bufs=1, space="PSUM")

    # ---- 1/Z as a rank-1 broadcast    # ---- rep_delta^T (step_rep-scaled) = (srep*W)^T @ R ----------------
    av_ps = psumB.tile([128, 4, 128], F32, name="av_ps", tag="b4", bufs=2)
    for p in range(NPAIR):
        for o in range(2):
            nc.tensor.matmul(
                av_ps[:, p, :], wps_bf[:, o, ts(p, 128)],
                R_bf[:, o, ds(128 * p, 128)],
                start=(o == 0), stop=(o == 1),
            )
    # rep2^T = repT + rz * av (diagonal quadrants only)
    rep2T = s4.tile([128, 4, NB], F32, name="rep2T")
    for h in range(2):
        rows = slice(64 * h, 64 * h + 64)
        nc.vector.tensor_tensor(
            rep2T[rows, :, :], av_ps[rows, :, ds(64 * h, 64)],
            rz_bc[rows, :, ds(64 * h, 64)], ALU.mult,
        )
    nc.vector.tensor_add(rep2T, rep2T, repT_f)
    rep2T_b = s4.tile([128, 4, NB], BF16, name="rep2T_b")
    nc.vector.tensor_copy(rep2T_b, rep2T)
    rep2T_bd = s4.tile([128, 4, 128], BF16, name="rep2T_bd")
    nc.vector.memset(rep2T_bd, 0.0)
    for h in range(2):
        rows = slice(64 * h, 64 * h + 64)
        nc.vector.tensor_scalar_mul(
            rep2T_bd[rows, :, ds(64 * h, 64)], rep2T[rows, :, :], SCALE
        )

    # ---- stage 2: self-attention among the 64 pooled tokens -------------
    r2_ps = psumB.tile([64, 4, 128], BF16, name="r2_ps", tag="sm", bufs=2)
    for p in range(NPAIR):
        nc.tensor.transpose(r2_ps[:, p, :], rep2T_b[:, p, :], ident_bf)
    r2_sb = s4.tile([64, 4, 128], BF16, name="r2_sb")
    nc.vector.tensor_copy(r2_sb, r2_ps)
    s2_ps = psumB.tile([128, 4, NB], F32, name="s2_ps", tag="sm", bufs=2)
    for p in range(NPAIR):
        nc.tensor.matmul(
            s2_ps[:, p, :], rep2T_bd[:, p, :], rep2T_b[:, p, :],
            start=True, stop=True,
        )
    p2_sb = s4.tile([128, 4, NB], BF16, name="p2_sb")
    nc.scalar.activation(
        out=p2_sb, in_=s2_ps, func=ACTF.Exp, bias=0.0, scale=1.0
    )
    z2 = s4.tile([128, 4], F32, name="z2")
    nc.vector.reduce_sum(z2, p2_sb, axis=AX.X)
    rz2 = s4.tile([128, 4], F32, name="rz2")
    nc.vector.reciprocal(rz2, z2)
    for p in range(NPAIR):
        nc.vector.tensor_scalar_mul(
            p2_sb[:, p, :], p2_sb[:, p, :], rz2[:, p : p + 1]
        )
    p2t_ps = psumB.tile([64, 4, 128], BF16, name="p2t_ps", tag="sm", bufs=2)
    for p in range(NPAIR):
        nc.tensor.transpose(p2t_ps[:, p, :], p2_sb[:, p, :], ident_bf)
    p2t_sb = s4.tile([64, 4, 128], BF16, name="p2t_sb")
    nc.vector.tensor_copy(p2t_sb, p2t_ps)
    # xd2^T = rep2^T @ P2n^T : [128 d-pair, 128 q-pair] per pair
    xd2t_ps = psumB.tile([128, 4, 128], F32, name="xd2t_ps", tag="b4", bufs=2)
    for p in range(NPAIR):
        nc.tensor.matmul(
            xd2t_ps[:, p, :], r2_sb[:, p, :], p2t_sb[:, p, :],
            start=True, stop=True,
        )
    # block-diagonal, 1/Z-scaled (the back-projection's softmax norm)
    xd2bd = s4.tile([128, 4, 128], BF16, name="xd2bd")
    nc.vector.memset(xd2bd, 0.0)
    for h in range(2):
        rows = slice(64 * h, 64 * h + 64)
        nc.vector.tensor_tensor(
            xd2bd[rows, :, ds(64 * h, 64)], xd2t_ps[rows, :, ds(64 * h, 64)],
            rz_bc[rows, :, ds(64 * h, 64)], ALU.mult,
        )
    # xd2W = xd2 @ (sx*W_out) : [128 q-pair, 256c] per pair
    xw_ps = psumB.tile([128, 4, C], F32, name="xw_ps", tag="xw", bufs=1)
    for p in range(NPAIR):
        nc.tensor.matmul(
            xw_ps[:, p, :], xd2bd[:, p, :], wos_bf[:, p, :],
            start=True, stop=True,
        )
    xd2w_sb = outp.tile([128, 4, 2, 128], BF16, name="xd2w_sb", tag="xdw", bufs=1)
    nc.vector.tensor_copy(xd2w_sb, xw_ps)
    s4.release()
    psumB.release()

    psumC = tc.alloc_tile_pool(name="psumC", bufs=1, space="PSUM")

    # ---- back-projection folded with the output projection --------------
    # out[c, t] = sum_pairs xd2W_pair[q, c]^T @ P_pair[q, t]  (+ b_out)
    for jq in range(4):
        for cs in range(2):
            op_ps = psumC.tile([128, 2, 512], F32, name="op_ps", tag="op", bufs=4)
            for p in range(NPAIR):
                for j2 in range(2):
                    nc.tensor.matmul(
                        op_ps[:, j2, :], xd2w_sb[:, p, cs, :],
                        p_tiles[p][:, ts(2 * jq + j2, 512)],
                        start=(p == 0), stop=(p == NPAIR - 1),
                        skip_group_check=True,
                    )
            for j2 in range(2):
                out_sb = outp.tile(
                    [128, 512], F32, name="out_sb", tag="out_sb", bufs=6
                )
                if j2 == 0:
                    nc.scalar.add(out_sb, op_ps[:, j2, :], bo_sb[:, cs : cs + 1])
                else:
                    nc.vector.tensor_scalar_add(
                        out_sb, op_ps[:, j2, :], bo_sb[:, cs : cs + 1]
                    )
                nc.sync.dma_start(out_r[:, cs, ts(2 * jq + j2, 512)], out_sb)
    psumC.release()
    pp.release()
    outp.release()
    consts.release()


_CACHE = {}


class _Runner:
    """Builds the Bass module once and keeps a single jitted shard_map
    executable alive, so repeat kernel() calls skip retracing/relowering."""

    def __init__(self):
        import jax
        import jax.numpy as jnp
        from jax.sharding import Mesh, PartitionSpec
        from jax.experimental.shard_map import shard_map
        from concourse import bass2jax

        self.jax = jax
        nc = build_module()
        self.nc = nc
        bass2jax.install_neuronx_cc_hook()

        partition_name = (
            nc.partition_id_tensor.name if nc.partition_id_tensor else None
        )
        in_names, out_names, out_avals = [], [], []
        for alloc in nc.m.functions[0].allocations:
            if not isinstance(alloc, mybir.MemoryLocationSet):
                continue
            name = alloc.memorylocations[0].name
            if alloc.kind == "ExternalInput":
                if name != partition_name:
                    in_names.append(name)
            elif alloc.kind == "ExternalOutput":
                out_names.append(name)
                out_avals.append(
                    jax.core.ShapedArray(
                        tuple(alloc.tensor_shape), mybir.dt.np(alloc.dtype)
                    )
                )
        n_params = len(in_names)
        n_outs = len(out_avals)
        all_names = list(in_names) + list(out_names)
        if partition_name is not None:
            all_names.append(partition_name)
        self.in_names = in_names
        self.out_names = out_names
        self.out_avals = out_avals

        def _body(*args):
            operands = list(args)
            if partition_name is not None:
                operands.append(bass2jax.partition_id_tensor())
            outs = bass2jax._bass_exec_p.bind(
                *operands,
                out_avals=tuple(out_avals),
                in_names=tuple(all_names),
                out_names=tuple(out_names),
                lowering_input_output_aliases=(),
                sim_require_finite=True,
                sim_require_nnan=True,
                nc=nc,
            )
            return tuple(outs)

        self.body = _body
        devices = jax.devices()[:B]
        mesh = Mesh(np.asarray(devices), ("core",))
        donate = tuple(range(n_params, n_params + n_outs))
        self.sharded = jax.jit(
            shard_map(
                _body, mesh=mesh,
                in_specs=(PartitionSpec("core"),) * (n_params + n_outs),
                out_specs=(PartitionSpec("core"),) * n_outs,
                check_rep=False,
            ),
            donate_argnums=donate,
            keep_unused=True,
        )

    def run(self, in_maps):
        concat_in = [
            np.concatenate([m[name] for m in in_maps], axis=0)
            for name in self.in_names
        ]
        zeros = [
            np.zeros((B * a.shape[0], *a.shape[1:]), a.dtype) for a in self.out_avals
        ]
        out_arrs = self.sharded(*concat_in, *zeros)
        return [
            {
                name: np.asarray(out_arrs[i]).reshape(B, *self.out_avals[i].shape)[c]
                for i, name in enumerate(self.out_names)
            }
            for c in range(B)
        ]

    def bench(self, in_maps, reps=8, inner=72, base=8):
        """Time device-resident executions (no donation, operands staged once).

        Times jitted chains of `base` and `inner` back-to-back kernel
        executions; returns (per_exec_seconds, base_chain_seconds, results)
        with per_exec = (t_inner - t_base) / (inner - base), which amortizes
        away the per-dispatch round-trip of this axon-tunneled environment.
        """
        import time
        from jax.sharding import Mesh, PartitionSpec, NamedSharding
        from jax.experimental.shard_map import shard_map

        jax = self.jax
        devices = jax.devices()[:B]
        mesh = Mesh(np.asarray(devices), ("core",))
        sharding = NamedSharding(mesh, PartitionSpec("core"))
        n_ops = len(self.in_names) + len(self.out_avals)

        def chain(n):
            def f(*args):
                outs = []
                for _ in range(n):
                    outs.extend(self.body(*args))
                return tuple(outs)
            return f

        concat_in = [
            np.concatenate([m[name] for m in in_maps], axis=0)
            for name in self.in_names
        ]
        zeros = [
            np.zeros((B * a.shape[0], *a.shape[1:]), a.dtype) for a in self.out_avals
        ]
        staged = [jax.device_put(a, sharding) for a in concat_in + zeros]

        times = {}
        out1 = None
        for n in (base, inner):
            jfn = jax.jit(
                shard_map(
                    chain(n), mesh=mesh,
                    in_specs=(PartitionSpec("core"),) * n_ops,
                    out_specs=(PartitionSpec("core"),) * (n * len(self.out_avals)),
                    check_rep=False,
                ),
                keep_unused=True,
            )
            out = jfn(*staged)
            jax.block_until_ready(out)
            best = float("inf")
            for _ in range(reps):
                t0 = time.perf_counter()
                out = jfn(*staged)
                jax.block_until_ready(out)
                best = min(best, time.perf_counter() - t0)
            times[n] = best
            if n == base:
                out1 = out
        per_exec = (times[inner] - times[base]) / (inner - base)
        if per_exec <= 0:
            per_exec = times[inner] / inner  # noise floor: report upper bound
        results = [
            {
                name: np.asarray(out1[i]).reshape(B, *self.out_avals[i].shape)[c]
                for i, name in enumerate(self.out_names)
            }
            for c in range(B)
        ]
        return per_exec, times[base], results


def _get_runner():
    key = CFG["ver"]
    if key not in _CACHE:
        _CACHE[key] = _Runner()
    return _CACHE[key]


def _make_in_maps(x, W_proj, step_rep, step_x, W_out, b_out):
    x = np.ascontiguousarray(np.asarray(x, dtype=np.float32))
    shared = {
        "w_proj": np.ascontiguousarray(np.asarray(W_proj, dtype=np.float32)),
        "w_out": np.ascontiguousarray(np.asarray(W_out, dtype=np.float32)),
        "b_out": np.ascontiguousarray(np.asarray(b_out, dtype=np.float32)),
        "s_rep": np.ascontiguousarray(
            np.asarray(step_rep, dtype=np.float32).reshape(HEADS)
        ),
        "s_x": np.ascontiguousarray(
            np.asarray(step_x, dtype=np.float32).reshape(HEADS)
        ),
    }
    return [
        {"x": np.ascontiguousarray(x[b].reshape(C, T)), **shared} for b in range(B)
    ]


def kernel(x, W_proj, step_rep, step_x, W_out, b_out):
    runner = _get_runner()
    results = runner.run(_make_in_maps(x, W_proj, step_rep, step_x, W_out, b_out))
    outs = [np.asarray(results[b]["out"]).reshape(C, 64, 64) for b in range(B)]
    return np.stack(outs, axis=0)
